# revision 78
# baseline (speedup 1.0000x reference)
"""Trainium2 Bass kernel for AdvancedIntegratedFiberOpticsNN.

Sharding: 8 cores = 4 images x 2 H-halves (pure data parallel; the only
cross-half quantity, avg_g, is recomputed per core from the full image).

Per-core device program (one TileContext):
  startup: the wdx slice of the weight buffer + cv go out first so phase-1
           matmuls start ~8us in; the first two stacked-x groups are
           prefetched ahead of the bulky wb tail; the 9 Sobel shift DMAs
           are spread over the sync/gpsimd/scalar queues.
  phase 0: Sobel gradient of the full image -> S_g -> per-channel scale
           s96, folded on-device into the pattern-conv and cls1 weights.
  phase 1: 3/5/7 input convs as 2 K=84 matmuls per 512-px flat chunk
           (4 column-shifted replicas of the row-stacked x in partitions
           0..83) -> bf16 ring (cls1 input) + fp8 shadow copies f8a/f8b
           (pattern-conv input; f8b is shifted +1 so odd-dx taps read at
           even addresses, a DoubleRow ISA requirement).
  phase 2: per 512-px chunk: 3x3 pattern conv as 5 dual-fp8 DoubleRow
           matmuls -- dy in {-1,0} pairs at stride WP=272 on f8a/f8b, and
           the three dy=+1 taps packed into 2 passes via a small sliding
           window P (one DMA per chunk pair) that holds dx-shifted copies
           at stride PW so {(1,-1),(1,0)} pair up -- then sigmoid,
           cls1 (K=96+72 -> 2x128), relu, cls2 (K=256 -> 128), relu,
           heads + anomaly-mean matmuls, output staged bf16 and streamed
           out per 8/16-row block as its chunks complete (DMA-cast f32).
  Phases 1 and 2 are software-pipelined chunk-by-chunk (LEAD=8). All
  per-chunk-pair DMAs (f8 casts, P window, output blocks) issue from the
  gpsimd queue so their semaphore waits never block the sync queue that
  feeds phase 1.
"""
import os
import numpy as np
import ml_dtypes

import concourse.bass as bass
import concourse.mybir as mybir
import concourse.tile as tile
from concourse import bacc
from concourse.bass_utils import run_bass_kernel_spmd

F32 = mybir.dt.float32
BF16 = mybir.dt.bfloat16
F8 = mybir.dt.float8e4
DR = mybir.MatmulPerfMode.DoubleRow
AF = mybir.ActivationFunctionType
ALU = mybir.AluOpType
BF = ml_dtypes.bfloat16

B, H, W = 4, 256, 256
R = 128                  # output rows per core
WP = 272                 # padded width (multiple of 16 for DoubleRow)
COL0 = 3                 # image col 0 lives at padded col 3
FR = 130                 # feature rows per core (R + 2)
FLAT = FR * WP           # 35360
CHUNK = 512
N1 = 70                  # phase-1 chunks (covers [0, 35840))
N2 = 68                  # phase-2 chunks (covers [WP, WP + 34816))
G = 16                   # fp8 feature buffer head guard
F8A_LEN = G + N1 * CHUNK          # 35856
F8B_LEN = G + N1 * CHUNK + 16     # 35872 (holds f[p-1] at p)
PW = 1040                # P window pitch (2 chunks): dy=+1, dx in {-1,0,1}
XP_ROWS = 136
XP_LEN = 8 + (XP_ROWS + 4) * WP + 8
SGG = 4                  # chunks per stacked-x DMA group
NSG = (N1 + SGG - 1) // SGG
OSH = 34 * CHUNK         # half of the output staging (64 rows)
LEAD = 8                 # phase-1 chunks traced ahead of phase 2
NB = 16                  # bf16 feature ring slots (> LEAD + 2)
RING_LEN = NB * CHUNK + WP   # tail pad mirrors next slot-0 head (wrap reads)

# packed bf16 weight buffer column offsets
WB_WDX, WB_PW, WB_W1F, WB_W1S, WB_W2, WB_WH, WB_ON = \
    0, 192, 1152, 1408, 1664, 1920, 1954
WB_LEN = 1984
PWS = 80                 # fp8 pattern-weight tap stride (16-aligned)

_NC_CACHE = {}
LAST_RESULTS = None      # BassKernelResults of the most recent run (for test.py)


# --------------------------------------------------------------------------
# host-side preparation
# --------------------------------------------------------------------------

def _host_prep(inp):
    x = np.asarray(inp['x'], np.float32)
    w3, b3 = np.asarray(inp['w3'], np.float32), np.asarray(inp['b3'], np.float32)
    w5, b5 = np.asarray(inp['w5'], np.float32), np.asarray(inp['b5'], np.float32)
    w7, b7 = np.asarray(inp['w7'], np.float32), np.asarray(inp['b7'], np.float32)
    grad_w = np.asarray(inp['grad_w'], np.float32)
    pos_w = np.asarray(inp['pos_w'], np.float32)
    grad_adj = float(np.asarray(inp['grad_adj']))
    pos_adj = float(np.asarray(inp['pos_adj']))
    npat = np.asarray(inp['normal_patterns'], np.float32)
    thr = np.asarray(inp['normal_thresholds'], np.float32)
    apat = np.asarray(inp['anomaly_patterns'], np.float32)
    w1 = np.asarray(inp['cls_w1'], np.float32)[:, :, 0, 0]
    b1 = np.asarray(inp['cls_b1'], np.float32)
    w2 = np.asarray(inp['cls_w2'], np.float32)[:, :, 0, 0]
    b2 = np.asarray(inp['cls_b2'], np.float32)
    rw = np.asarray(inp['region_w'], np.float32)[:, :, 0, 0]
    rb = np.asarray(inp['region_b'], np.float32)
    aw = np.asarray(inp['anom_w'], np.float32)[:, :, 0, 0]
    ab = np.asarray(inp['anom_b'], np.float32)
    qw = np.asarray(inp['qual_w'], np.float32)[:, :, 0, 0]
    qb = np.asarray(inp['qual_b'], np.float32)

    ypos = np.linspace(-1.0, 1.0, H, dtype=np.float32).reshape(H, 1)
    xpos = np.linspace(-1.0, 1.0, W, dtype=np.float32).reshape(1, W)
    avg_p = float(np.sqrt(xpos ** 2 + ypos ** 2).mean())
    posf = 1.0 + pos_w * avg_p * pos_adj
    br_of = np.repeat(np.arange(3), 32)
    A_vec = posf[br_of].astype(np.float32)
    Bv_vec = (posf * grad_w * grad_adj / (3.0 * H * W))[br_of].astype(np.float32)

    # input-conv weights, dx-major: wdx[(r*3+c), dxi, co]
    wdx = np.zeros((21, 7, 96), np.float32)
    for co_base, wbr, k2 in ((0, w3, 1), (32, w5, 2), (64, w7, 3)):
        for r in range(7):
            dy = r - 3
            if abs(dy) > k2:
                continue
            for c in range(3):
                for dxi in range(7):
                    dx = dxi - 3
                    if abs(dx) > k2:
                        continue
                    wdx[r * 3 + c, dxi, co_base:co_base + 32] = \
                        wbr[:, c, dy + k2, dx + k2]
    # K=84 layout: partition (g*21 + r*3 + c) holds x shifted by (r rows,
    # g cols); matmul m covers taps dxi = 4m + g
    wdx2 = np.zeros((84, 2, 96), np.float32)
    for g in range(4):
        for m in range(2):
            dxi = 4 * m + g
            if dxi <= 6:
                wdx2[g * 21:(g + 1) * 21, m, :] = wdx[:, dxi, :]

    pat = np.concatenate([npat, apat], axis=0)          # [72, 96, 3, 3]
    # 5-pass DoubleRow slot pairs (order matches the rhs tile-pair order):
    #   p1 {(-1,0),(0,0)} on f8a   p2 {(-1,-1),(0,-1)} / p3 {(-1,1),(0,1)}
    #   on the Q window            p4 {(1,-1),(1,0)} / p5 {(1,1), dead} on P
    pw = np.zeros((96, 10, PWS), np.float32)
    taps = [(-1, 0), (0, 0), (-1, -1), (0, -1), (-1, 1), (0, 1),
            (1, -1), (1, 0), (1, 1)]
    for s, (dy, dx) in enumerate(taps):
        pw[:, s, 0:72] = pat[:, :, dy + 1, dx + 1].T

    w1f = np.ascontiguousarray(w1[:, 0:96].T)           # [96, 256]
    w1s = np.ascontiguousarray(w1[:, 96:168].T)         # [72, 256]
    w2k = np.concatenate([w2[:, 0:128].T, w2[:, 128:256].T], axis=1)  # [128,256]
    # head channels on-chip: partitions 0..2 = region, 32 = anom, 33 = qual
    wh = np.zeros((128, 34), np.float32)
    wh[:, 0:3] = rw.T
    wh[:, 32] = aw[0]
    wh[:, 33] = qw[0]

    # one packed bf16 weight buffer [128, WB_LEN]
    wb = np.zeros((128, WB_LEN), BF)
    wb[0:84, WB_WDX:WB_WDX + 192] = wdx2.reshape(84, 192).astype(BF)
    wb[0:96, WB_PW:WB_PW + 800] = pw.reshape(96, 800).astype(BF)
    wb[0:96, WB_W1F:WB_W1F + 256] = w1f.astype(BF)
    wb[0:72, WB_W1S:WB_W1S + 256] = w1s.astype(BF)
    wb[:, WB_W2:WB_W2 + 256] = w2k.astype(BF)
    wb[:, WB_WH:WB_WH + 34] = wh.astype(BF)
    wb[0:48, WB_ON] = np.full(48, -1.0 / 48.0, BF)

    # one packed f32 constant buffer [128, 16]
    cv = np.zeros((128, 16), np.float32)
    cv[0:96, 0] = np.concatenate([b3, b5, b7])            # fbias
    cv[0:48, 1] = thr
    cv[:, 2] = b1[0:128]
    cv[:, 3] = b1[128:256]
    cv[:, 4] = b2
    cv[0:3, 5] = rb
    cv[32, 5] = ab[0]
    cv[33, 5] = qb[0]
    cv[0:96, 6] = A_vec
    cv[0:96, 7] = Bv_vec

    shared = {'wb': wb, 'cv': cv}

    cores = []
    for i in range(8):
        b, half = i // 2, i % 2
        r0 = R * half
        xp = np.zeros((3, XP_LEN), BF)
        body = np.zeros((3, XP_ROWS + 4, WP), np.float32)
        y0, y1 = max(0, r0 - 4), min(H, r0 - 4 + XP_ROWS)
        body[:, y0 - (r0 - 4):y1 - (r0 - 4), COL0:COL0 + W] = x[b, :, y0:y1, :]
        xp[:, 8:8 + (XP_ROWS + 4) * WP] = body.reshape(3, -1).astype(BF)
        xf = np.zeros((3, 258, WP), BF)                   # zero rows 0, 257
        xf[:, 1:257, COL0:COL0 + W] = x[b].astype(BF)
        cvi = shared['cv'].copy()
        cvi[0, 8] = 1.0 if r0 > 0 else 0.0                # row masks
        cvi[0, 9] = 1.0 if r0 + R < H else 0.0
        cores.append(dict(xp=xp, xf=xf, cv=cvi, b=b, r0=r0))
    return shared, cores


# --------------------------------------------------------------------------
# device program
# --------------------------------------------------------------------------

def _build_nc():
    nc = bacc.Bacc(None, target_bir_lowering=False, debug=False)

    xp_t = nc.declare_dram_parameter("xp", [3, XP_LEN], BF16, isOutput=False)
    xf_t = nc.declare_dram_parameter("xf", [3, 258, WP], BF16, isOutput=False)
    wb_t = nc.declare_dram_parameter("wb", [128, WB_LEN], BF16, isOutput=False)
    cv_t = nc.declare_dram_parameter("cv", [128, 16], F32, isOutput=False)
    out_t = nc.declare_dram_parameter("out", [5, R, W], F32, isOutput=True)

    with tile.TileContext(nc) as tc:
        import contextlib
        with contextlib.ExitStack() as ctx:
            consts = ctx.enter_context(tc.tile_pool(name="consts", bufs=1))
            big = ctx.enter_context(tc.tile_pool(name="big", bufs=1))
            sgp = ctx.enter_context(tc.tile_pool(name="sgp", bufs=4))
            work = ctx.enter_context(tc.tile_pool(name="work", bufs=3))
            sap = ctx.enter_context(tc.tile_pool(name="sap", bufs=4))
            qpp = ctx.enter_context(tc.tile_pool(name="qpp", bufs=4))
            sob_cm = tc.tile_pool(name="sob", bufs=1)
            sob = sob_cm.__enter__()

            # ---- urgent constants first: the wdx slice of wb (phase-1
            # matmuls) and cv (phase-1 activation bias) ----
            wb_sb = consts.tile([128, WB_LEN], BF16)
            nc.sync.dma_start(out=wb_sb[:, 0:WB_PW], in_=wb_t[:, 0:WB_PW])
            cv_sb = consts.tile([128, 16], F32)
            nc.sync.dma_start(out=cv_sb, in_=cv_t[:, :])

            # ---- phase 0a: gray planes via shifted-copy DMAs ----
            g0 = sob.tile([128, 2, WP], BF16)    # gray rows r
            gu = sob.tile([128, 2, WP], BF16)    # gray rows r+1
            gd = sob.tile([128, 2, WP], BF16)    # gray rows r-1
            xsh = sob.tile([128, 3, 2, 3, WP], BF16)  # [p, shift, blk, c, w]
            for k, (sh, c) in enumerate([(s, c) for s in range(3)
                                         for c in range(3)]):
                in_ap = bass.AP(
                    tensor=xf_t[:, :, :].tensor,
                    offset=c * 258 * WP + sh * WP,
                    ap=[[WP, 128], [128 * WP, 2], [1, WP]])
                eng = (nc.sync, nc.gpsimd, nc.scalar)[k % 3]
                eng.dma_start(out=xsh[:, sh, :, c, :], in_=in_ap)
            for sh, gt in ((1, g0), (2, gu), (0, gd)):
                nc.vector.tensor_add(gt, xsh[:, sh, :, 0, :],
                                     xsh[:, sh, :, 1, :])
                nc.vector.tensor_add(gt, gt, xsh[:, sh, :, 2, :])

            # stacked-x group loader (phase 1 feed); prefetch the first two
            # groups before the bulky tail of wb lands on the sync queue
            sc_tiles = {}

            def load_sc(g):
                if g in sc_tiles:
                    return sc_tiles[g]
                gw = min(SGG, N1 - g * SGG) * CHUNK + 6
                sc = sgp.tile([84, SGG * CHUNK + 8], BF16,
                              name=f"sc{g}", tag="sc")
                for rg in range(4):
                    in_ap = bass.AP(
                        tensor=xp_t[:, :].tensor,
                        offset=8 + g * SGG * CHUNK - 3 + rg,
                        ap=[[WP, 7], [XP_LEN, 3], [1, gw]])
                    nc.sync.dma_start(
                        out=sc[21 * rg:21 * (rg + 1), 0:gw], in_=in_ap)
                sc_tiles[g] = sc
                return sc

            load_sc(0)
            load_sc(1)
            # the rest of the weight buffer (pattern/cls weights; not needed
            # until the phase-0 tail folds and phase 2)
            nc.sync.dma_start(out=wb_sb[:, WB_PW:], in_=wb_t[:, WB_PW:])
            wdx_sb = wb_sb[0:84, WB_WDX:WB_WDX + 192].rearrange(
                "p (m k) -> p m k", m=2)
            pw_sb = wb_sb[0:96, WB_PW:WB_PW + 800]
            w1f_sb = wb_sb[0:96, WB_W1F:WB_W1F + 256]
            w1s_sb = wb_sb[0:72, WB_W1S:WB_W1S + 256]
            w2_sb = wb_sb[:, WB_W2:WB_W2 + 256]
            wh_sb = wb_sb[:, WB_WH:WB_WH + 34]
            on48_sb = wb_sb[0:48, WB_ON:WB_ON + 1]
            fb_sb = cv_sb[0:96, 0:1]
            thr_sb = cv_sb[0:48, 1:2]
            b1_sb = cv_sb[:, 2:4]
            b2_sb = cv_sb[:, 4:5]
            bh_sb = cv_sb[0:34, 5:6]
            av_sb = cv_sb[0:96, 6:7]
            bv_sb = cv_sb[0:96, 7:8]
            msk_sb = cv_sb[0:1, 8:10]
            ones96 = consts.tile([1, 96], F32)
            nc.vector.memset(ones96, 1.0)
            ones128 = consts.tile([128, 1], F32)
            nc.vector.memset(ones128, 1.0)
            pwf8 = consts.tile([96, 10 * PWS], F8)     # folded pattern w (fp8)
            w1ff_sb = consts.tile([96, 256], BF16)     # folded cls1 feat weights
            m96 = consts.tile([96, 2], F32)            # row masks broadcast
            s96 = consts.tile([96, 1], F32)            # feature scale vector
            sgsc = consts.tile([1, 1], F32)            # S_g scalar

            fring = big.tile([96, RING_LEN], BF16)     # bf16 features (cls1)
            f8a = big.tile([96, F8A_LEN], F8)          # fp8 features (pattern)
            f8b = big.tile([96, F8B_LEN], F8)          # f8b[p] = f[p-1]

            with tc.tile_pool(name="ps0", bufs=1, space="PSUM") as ps0:
                # ---- phase 0b: Sobel from the gray planes (DVE only) ----
                Dt = sob.tile([128, 2, WP], BF16)
                nc.vector.tensor_sub(Dt, gu, gd)
                gy = sob.tile([128, 2, 256], BF16)
                q2 = sob.tile([128, 2, 256], BF16)
                nc.vector.tensor_add(gy, Dt[:, :, 2:258], Dt[:, :, 4:260])
                nc.vector.tensor_scalar_mul(q2, Dt[:, :, 3:259], 2.0)
                nc.vector.tensor_add(gy, gy, q2)
                cd0 = sob.tile([128, 2, 256], BF16)
                nc.vector.tensor_sub(cd0, g0[:, :, 4:260], g0[:, :, 2:258])
                gx = sob.tile([128, 2, 256], BF16)
                nc.vector.tensor_scalar_mul(gx, cd0, 2.0)
                nc.vector.tensor_sub(cd0, gu[:, :, 4:260], gu[:, :, 2:258])
                nc.vector.tensor_add(gx, gx, cd0)
                nc.vector.tensor_sub(cd0, gd[:, :, 4:260], gd[:, :, 2:258])
                nc.vector.tensor_add(gx, gx, cd0)
                nc.vector.tensor_mul(gx, gx, gx)
                nc.vector.tensor_mul(gy, gy, gy)
                nc.vector.tensor_add(gx, gx, gy)
                gsc = sob.tile([128, 2, 256], F32)
                rs = sob.tile([128, 1], F32)
                # broadcast row masks now (prologue chunk 0 reads m96)
                scr = ps0.tile([96, 2], F32, tag="scr")
                nc.tensor.matmul(scr, lhsT=ones96, rhs=msk_sb,
                                 start=True, stop=True)
                nc.vector.tensor_copy(m96, scr)

            nc.vector.memset(f8a[:, 0:G], 0.0)
            nc.vector.memset(f8b[:, 0:G + 1], 0.0)

            with tc.tile_pool(name="ppp", bufs=2, space="PSUM") as ppp, \
                 tc.tile_pool(name="ps1p", bufs=1, space="PSUM") as ps1p, \
                 tc.tile_pool(name="ps2p", bufs=1, space="PSUM") as ps2p, \
                 tc.tile_pool(name="php", bufs=1, space="PSUM") as php, \
                 tc.tile_pool(name="pfp", bufs=2, space="PSUM") as pfp:

                tail_st = {}

                def phase0_tail_a():
                    # sqrt + S_g partition-sum; scratch lives on the heads
                    # tag (no other "ph" allocations until tail_b reads it)
                    nc.scalar.activation(gsc, gx, AF.Sqrt, accum_out=rs)
                    scr2 = php.tile([96, 8], F32, name="scr2", tag="ph")
                    nc.tensor.matmul(scr2[0:1, 0:1], lhsT=rs, rhs=ones128,
                                     start=True, stop=True)
                    tail_st['scr2'] = scr2

                def phase0_tail_b():
                    scr2 = tail_st['scr2']
                    nc.vector.tensor_copy(sgsc, scr2[0:1, 0:1])
                    nc.tensor.matmul(scr2[:, 2:3], lhsT=ones96, rhs=sgsc,
                                     start=True, stop=True)
                    nc.vector.tensor_scalar(s96, scr2[:, 2:3], bv_sb, av_sb,
                                            op0=ALU.mult, op1=ALU.add)
                    nc.vector.tensor_scalar_mul(pwf8, pw_sb, s96)
                    nc.vector.tensor_scalar_mul(w1ff_sb, w1f_sb, s96)

                ph1_pend = {}

                def phase1_mm(i):
                    # phase-1 matmuls only (PE queue); the drain is issued
                    # separately so mid-chunk interleaving never inserts a
                    # Scalar op between the phase-2 drains
                    g, qq = divmod(i, SGG)
                    sc = load_sc(g)
                    pf = pfp.tile([96, CHUNK], F32, name=f"pf{i}", tag="pf")
                    for m in range(2):
                        nc.tensor.matmul(
                            pf, lhsT=wdx_sb[:, m, :],
                            rhs=sc[:, qq * CHUNK + 4 * m:
                                   qq * CHUNK + 4 * m + CHUNK],
                            start=(m == 0), stop=(m == 1))
                    ph1_pend[i] = pf

                def phase1_drain(i):
                    pf = ph1_pend.pop(i)
                    s = i * CHUNK
                    slot = (i % NB) * CHUNK
                    nc.scalar.activation(
                        out=fring[:, slot:slot + CHUNK], in_=pf,
                        func=AF.Identity, bias=fb_sb, scale=1.0)
                    if i % NB == 0 and i > 0:
                        # mirror slot-0 head at ring tail for wrap-crossing
                        # cls1 reads
                        nc.scalar.activation(
                            out=fring[:, NB * CHUNK:NB * CHUNK + WP],
                            in_=pf[:, 0:WP],
                            func=AF.Identity, bias=fb_sb, scale=1.0)
                    # fp8 shadows for the 3x3 pattern conv (casting DMAs --
                    # data moves on the DMA hw, not a compute engine).
                    # Copy chunk pairs: slots i-1, i are ring-contiguous for
                    # odd i (wrap happens at even i).
                    if i % 2 == 1:
                        s0, sl0 = s - CHUNK, slot - CHUNK
                        nc.gpsimd.dma_start(
                            out=f8a[:, G + s0:G + s0 + 2 * CHUNK],
                            in_=fring[:, sl0:sl0 + 2 * CHUNK])
                        nc.gpsimd.dma_start(
                            out=f8b[:, G + s0 + 1:G + s0 + 2 * CHUNK + 1],
                            in_=fring[:, sl0:sl0 + 2 * CHUNK])
                        # re-zero the pattern-conv pad columns of the rows
                        # touched by the pair copy above
                        row_lo = (s - CHUNK) // WP
                        row_hi = min((s + CHUNK - 1) // WP, FR - 1)
                        fva = f8a[:, G + row_lo * WP:
                                  G + (row_hi + 1) * WP].rearrange(
                            "p (r w) -> p r w", w=WP)
                        nc.gpsimd.memset(fva[:, :, 2:3], 0.0)
                        nc.gpsimd.memset(fva[:, :, 259:260], 0.0)
                        fvb = f8b[:, G + row_lo * WP + 1:
                                  G + (row_hi + 1) * WP + 1].rearrange(
                            "p (r w) -> p r w", w=WP)
                        nc.gpsimd.memset(fvb[:, :, 2:3], 0.0)
                        nc.gpsimd.memset(fvb[:, :, 259:260], 0.0)
                    if i == 1:
                        nc.vector.tensor_scalar_mul(
                            f8a[:, G:G + WP], f8a[:, G:G + WP], m96[:, 0:1])
                        nc.vector.tensor_scalar_mul(
                            f8b[:, G + 1:G + WP + 1],
                            f8b[:, G + 1:G + WP + 1], m96[:, 0:1])
                    if i == N1 - 1:
                        r129 = G + 129 * WP
                        nc.vector.tensor_scalar_mul(
                            f8a[:, r129:r129 + WP],
                            f8a[:, r129:r129 + WP], m96[:, 1:2])
                        nc.vector.tensor_scalar_mul(
                            f8b[:, r129 + 1:r129 + WP + 1],
                            f8b[:, r129 + 1:r129 + WP + 1], m96[:, 1:2])

                def phase1_chunk(i):
                    phase1_mm(i)
                    phase1_drain(i)

                def phase2_chunk(j, os_t, jl, mid=None):
                    t = WP + j * CHUNK
                    rslot = (j % NB) * CHUNK + WP
                    if j % 2 == 0:
                        # sliding fp8 tap window for the dy=+1 row (3 dx
                        # copies); dx=+-1 of dy in {-1,0} come from f8b
                        pt = qpp.tile([96, 3, PW], F8, name=f"p8{j}", tag="p8")
                        nc.gpsimd.dma_start(out=pt, in_=bass.AP(
                            tensor=f8a.tensor,
                            offset=f8a.offset + G + t + WP - 1,
                            ap=[[f8a.ap[0][0], 96], [1, 3], [1, PW]]))
                        phase2_chunk.pt = pt
                    pt = phase2_chunk.pt
                    qo = (j % 2) * CHUNK
                    pp = ppp.tile([72, CHUNK], F32, name=f"pp{j}", tag="pp")
                    pv = pwf8.rearrange("p (t m) -> p t m", t=10)
                    # 5 dual-fp8 DoubleRow passes
                    passes = (
                        (f8a, G + t - WP, WP),            # (-1,0),(0,0)
                        (f8b, G + t - WP, WP),            # (-1,-1),(0,-1)
                        (f8b, G + t - WP + 2, WP),        # (-1,1),(0,1)
                        (pt, qo, PW),                     # (1,-1),(1,0)
                        (pt, 2 * PW + qo, 16),            # (1,1), dead
                    )
                    for k, (buf, off, stride) in enumerate(passes):
                        rhs = bass.AP(
                            tensor=buf.tensor, offset=buf.offset + off,
                            ap=[[buf.ap[0][0], 96], [stride, 2], [1, CHUNK]])
                        nc.tensor.matmul(
                            pp, lhsT=pv[:, 2 * k:2 * k + 2, 0:72],
                            rhs=rhs, start=(k == 0), stop=(k == 4),
                            perf_mode=DR)
                    sa = sap.tile([72, CHUNK], BF16, name=f"sa{j}", tag="sa")
                    nc.scalar.activation(sa, pp, AF.Sigmoid)
                    an = sap.tile([48, CHUNK], BF16, name=f"an{j}", tag="an")
                    nc.vector.tensor_scalar(an, sa[0:48, :], thr_sb, 0.0,
                                            op0=ALU.subtract, op1=ALU.min)
                    ps1 = ps1p.tile([128, 2 * CHUNK], F32, name=f"ps1_{j}",
                                    tag="ps1")
                    for m in range(2):
                        nc.tensor.matmul(
                            ps1[:, m * CHUNK:(m + 1) * CHUNK],
                            lhsT=w1ff_sb[:, m * 128:(m + 1) * 128],
                            rhs=fring[:, rslot:rslot + CHUNK],
                            start=True, stop=False)
                        nc.tensor.matmul(
                            ps1[:, m * CHUNK:(m + 1) * CHUNK],
                            lhsT=w1s_sb[:, m * 128:(m + 1) * 128],
                            rhs=sa, start=False, stop=True)
                    h = work.tile([128, 2 * CHUNK], BF16, name=f"h{j}", tag="h")
                    nc.scalar.activation(h[:, 0:CHUNK], ps1[:, 0:CHUNK],
                                         AF.Relu, bias=b1_sb[:, 0:1])
                    nc.vector.tensor_scalar(h[:, CHUNK:], ps1[:, CHUNK:],
                                            b1_sb[:, 1:2], 0.0,
                                            op0=ALU.add, op1=ALU.max)
                    if mid is not None:
                        # independent phase-1 matmuls (PE-only) fill the PE
                        # while the h drains complete
                        mid()
                    ps2 = ps2p.tile([128, CHUNK], F32, name=f"ps2_{j}",
                                    tag="ps2")
                    nc.tensor.matmul(ps2, lhsT=w2_sb[:, 0:128],
                                     rhs=h[:, 0:CHUNK], start=True, stop=False)
                    nc.tensor.matmul(ps2, lhsT=w2_sb[:, 128:256],
                                     rhs=h[:, CHUNK:], start=False, stop=True)
                    p = work.tile([128, CHUNK], BF16, name=f"p{j}", tag="p")
                    nc.vector.tensor_scalar(p, ps2, b2_sb, 0.0,
                                            op0=ALU.add, op1=ALU.max)
                    # heads share one PSUM bank: rows 0..33 heads, 64 mean
                    pht = php.tile([66, CHUNK], F32, name=f"ph{j}", tag="ph")
                    nc.tensor.matmul(pht[0:34, :], lhsT=wh_sb, rhs=p,
                                     start=True, stop=True)
                    nc.tensor.matmul(pht[64:65, :], lhsT=on48_sb, rhs=an,
                                     start=True, stop=True)
                    osl = os_t[:, jl * CHUNK:(jl + 1) * CHUNK]
                    nc.scalar.activation(osl[32:34, :], pht[32:34, :],
                                         AF.Sigmoid, bias=bh_sb[32:34, :])
                    nc.vector.tensor_add(osl[32:33, :], osl[32:33, :],
                                         pht[64:65, :])
                    nc.scalar.activation(osl[0:3, :], pht[0:3, :],
                                         AF.Identity, bias=bh_sb[0:3, :])

                for i in range(5):
                    phase1_chunk(i)
                phase0_tail_a()
                for i in range(5, LEAD):
                    phase1_chunk(i)
                phase0_tail_b()
                sob_cm.__exit__(None, None, None)
                osp = ctx.enter_context(tc.tile_pool(name="osp", bufs=2))
                # stream each 16-row output block as soon as its chunks
                # complete (bf16->f32 casting DMAs), so the final DMAs
                # don't all pile up after the last chunk
                blk = {8: (0, 16), 16: (16, 16), 25: (32, 16),
                       29: (48, 8), 33: (56, 8)}
                for hh in range(2):
                    os_t = osp.tile([34, OSH], BF16, name=f"os{hh}", tag="os")
                    ov = os_t.rearrange("p (r w) -> p r w", w=WP)
                    for jl in range(34):
                        j = hh * 34 + jl
                        d = j + LEAD - 1
                        if j >= 1 and LEAD <= d < N1:
                            phase1_drain(d)
                        mid = (lambda i=j + LEAD: phase1_mm(i)) \
                            if j + LEAD < N1 else None
                        phase2_chunk(j, os_t, jl, mid)
                        if jl in blk:
                            rr, nr = blk[jl]
                            nc.gpsimd.dma_start(
                                out=out_t[0:3,
                                          hh * 64 + rr:hh * 64 + rr + nr, :],
                                in_=ov[0:3, rr:rr + nr, COL0:COL0 + W])
                            nc.gpsimd.dma_start(
                                out=out_t[3:5,
                                          hh * 64 + rr:hh * 64 + rr + nr, :],
                                in_=ov[32:34, rr:rr + nr, COL0:COL0 + W])
    nc.compile()
    return nc


def _get_nc():
    if 'nc' not in _NC_CACHE:
        _NC_CACHE['nc'] = _build_nc()
    return _NC_CACHE['nc']


# --------------------------------------------------------------------------
# entry point
# --------------------------------------------------------------------------

def kernel(**inputs) -> np.ndarray:
    global LAST_RESULTS
    nc = _get_nc()
    shared, cores = _host_prep(inputs)
    in_maps = []
    for ci in cores:
        in_maps.append({'wb': shared['wb'], 'xp': ci['xp'], 'xf': ci['xf'],
                        'cv': ci['cv']})
    res = run_bass_kernel_spmd(nc, in_maps, core_ids=list(range(8)),
                               trace=bool(os.environ.get("BASS_TRACE")))
    LAST_RESULTS = res
    full = np.zeros((B, 5, H, W), np.float32)
    for i, ci in enumerate(cores):
        full[ci['b'], :, ci['r0']:ci['r0'] + R, :] = res.results[i]['out']
    return full



# revision 82
# speedup vs baseline: 1.0376x; 1.0376x over previous
"""Trainium2 Bass kernel for AdvancedIntegratedFiberOpticsNN.

Sharding: 8 cores = 4 images x 2 H-halves (pure data parallel; the only
cross-half quantity, avg_g, is recomputed per core from the full image).

Per-core device program (one TileContext):
  startup: the wdx slice of the weight buffer + cv go out first so phase-1
           matmuls start ~8us in; the first two stacked-x groups are
           prefetched ahead of the bulky wb tail; the 9 Sobel shift DMAs
           are spread over the sync/gpsimd/scalar queues.
  phase 0: Sobel gradient of the full image -> S_g -> per-channel scale
           s96, folded on-device into the pattern-conv and cls1 weights.
  phase 1: 3/5/7 input convs as 2 K=84 matmuls per 512-px flat chunk
           (4 column-shifted replicas of the row-stacked x in partitions
           0..83) -> bf16 ring (cls1 input) + fp8 shadow copies f8a/f8b
           (pattern-conv input; f8b is shifted +1 so odd-dx taps read at
           even addresses, a DoubleRow ISA requirement).
  phase 2: per 512-px chunk: 3x3 pattern conv as 5 dual-fp8 DoubleRow
           matmuls -- dy in {-1,0} pairs at stride WP=272 on f8a/f8b, and
           the three dy=+1 taps packed into 2 passes via a small sliding
           window P (one DMA per chunk pair) that holds dx-shifted copies
           at stride PW so {(1,-1),(1,0)} pair up -- then sigmoid,
           cls1 (K=96+72 -> 2x128), relu, cls2 (K=256 -> 128), relu,
           heads + anomaly-mean matmuls, output staged bf16 and streamed
           out per 8/16-row block as its chunks complete (DMA-cast f32).
  Phases 1 and 2 are software-pipelined chunk-by-chunk (LEAD=8). All
  per-chunk-pair DMAs (f8 casts, P window, output blocks) issue from the
  gpsimd queue so their semaphore waits never block the sync queue that
  feeds phase 1.
"""
import os
import numpy as np
import ml_dtypes

import concourse.bass as bass
import concourse.mybir as mybir
import concourse.tile as tile
from concourse import bacc
from concourse.bass_utils import run_bass_kernel_spmd

F32 = mybir.dt.float32
BF16 = mybir.dt.bfloat16
F8 = mybir.dt.float8e4
DR = mybir.MatmulPerfMode.DoubleRow
AF = mybir.ActivationFunctionType
ALU = mybir.AluOpType
BF = ml_dtypes.bfloat16

B, H, W = 4, 256, 256
R = 128                  # output rows per core
WP = 272                 # padded width (multiple of 16 for DoubleRow)
COL0 = 3                 # image col 0 lives at padded col 3
FR = 130                 # feature rows per core (R + 2)
FLAT = FR * WP           # 35360
CHUNK = 512
N1 = 70                  # phase-1 chunks (covers [0, 35840))
N2 = 68                  # phase-2 chunks (covers [WP, WP + 34816))
G = 16                   # fp8 feature buffer head guard
F8A_LEN = G + N1 * CHUNK          # 35856
F8B_LEN = G + N1 * CHUNK + 16     # 35872 (holds f[p-1] at p)
PW = 1040                # P window pitch (2 chunks): dy=+1, dx in {-1,0,1}
XP_ROWS = 136
XP_LEN = 8 + (XP_ROWS + 4) * WP + 8
SGG = 4                  # chunks per stacked-x DMA group
NSG = (N1 + SGG - 1) // SGG
OSH = 34 * CHUNK         # half of the output staging (64 rows)
LEAD = 8                 # phase-1 chunks traced ahead of phase 2
NB = 16                  # bf16 feature ring slots (> LEAD + 2)
RING_LEN = NB * CHUNK + WP   # tail pad mirrors next slot-0 head (wrap reads)

# packed bf16 weight buffer column offsets
WB_WDX, WB_PW, WB_W1F, WB_W1S, WB_W2, WB_WH, WB_ON = \
    0, 192, 1152, 1408, 1664, 1920, 1954
WB_LEN = 1984
PWS = 80                 # fp8 pattern-weight tap stride (16-aligned)

_NC_CACHE = {}
LAST_RESULTS = None      # BassKernelResults of the most recent run (for test.py)


# --------------------------------------------------------------------------
# host-side preparation
# --------------------------------------------------------------------------

def _host_prep(inp):
    x = np.asarray(inp['x'], np.float32)
    w3, b3 = np.asarray(inp['w3'], np.float32), np.asarray(inp['b3'], np.float32)
    w5, b5 = np.asarray(inp['w5'], np.float32), np.asarray(inp['b5'], np.float32)
    w7, b7 = np.asarray(inp['w7'], np.float32), np.asarray(inp['b7'], np.float32)
    grad_w = np.asarray(inp['grad_w'], np.float32)
    pos_w = np.asarray(inp['pos_w'], np.float32)
    grad_adj = float(np.asarray(inp['grad_adj']))
    pos_adj = float(np.asarray(inp['pos_adj']))
    npat = np.asarray(inp['normal_patterns'], np.float32)
    thr = np.asarray(inp['normal_thresholds'], np.float32)
    apat = np.asarray(inp['anomaly_patterns'], np.float32)
    w1 = np.asarray(inp['cls_w1'], np.float32)[:, :, 0, 0]
    b1 = np.asarray(inp['cls_b1'], np.float32)
    w2 = np.asarray(inp['cls_w2'], np.float32)[:, :, 0, 0]
    b2 = np.asarray(inp['cls_b2'], np.float32)
    rw = np.asarray(inp['region_w'], np.float32)[:, :, 0, 0]
    rb = np.asarray(inp['region_b'], np.float32)
    aw = np.asarray(inp['anom_w'], np.float32)[:, :, 0, 0]
    ab = np.asarray(inp['anom_b'], np.float32)
    qw = np.asarray(inp['qual_w'], np.float32)[:, :, 0, 0]
    qb = np.asarray(inp['qual_b'], np.float32)

    ypos = np.linspace(-1.0, 1.0, H, dtype=np.float32).reshape(H, 1)
    xpos = np.linspace(-1.0, 1.0, W, dtype=np.float32).reshape(1, W)
    avg_p = float(np.sqrt(xpos ** 2 + ypos ** 2).mean())
    posf = 1.0 + pos_w * avg_p * pos_adj
    br_of = np.repeat(np.arange(3), 32)
    A_vec = posf[br_of].astype(np.float32)
    Bv_vec = (posf * grad_w * grad_adj / (3.0 * H * W))[br_of].astype(np.float32)

    # input-conv weights, dx-major: wdx[(r*3+c), dxi, co]
    wdx = np.zeros((21, 7, 96), np.float32)
    for co_base, wbr, k2 in ((0, w3, 1), (32, w5, 2), (64, w7, 3)):
        for r in range(7):
            dy = r - 3
            if abs(dy) > k2:
                continue
            for c in range(3):
                for dxi in range(7):
                    dx = dxi - 3
                    if abs(dx) > k2:
                        continue
                    wdx[r * 3 + c, dxi, co_base:co_base + 32] = \
                        wbr[:, c, dy + k2, dx + k2]
    # K=84 layout: partition (g*21 + r*3 + c) holds x shifted by (r rows,
    # g cols); matmul m covers taps dxi = 4m + g
    wdx2 = np.zeros((84, 2, 96), np.float32)
    for g in range(4):
        for m in range(2):
            dxi = 4 * m + g
            if dxi <= 6:
                wdx2[g * 21:(g + 1) * 21, m, :] = wdx[:, dxi, :]

    pat = np.concatenate([npat, apat], axis=0)          # [72, 96, 3, 3]
    # 5-pass DoubleRow slot pairs (order matches the rhs tile-pair order):
    #   p1 {(-1,0),(0,0)} on f8a   p2 {(-1,-1),(0,-1)} / p3 {(-1,1),(0,1)}
    #   on the Q window            p4 {(1,-1),(1,0)} / p5 {(1,1), dead} on P
    pw = np.zeros((96, 10, PWS), np.float32)
    taps = [(-1, 0), (0, 0), (-1, -1), (0, -1), (-1, 1), (0, 1),
            (1, -1), (1, 0), (1, 1)]
    for s, (dy, dx) in enumerate(taps):
        pw[:, s, 0:72] = pat[:, :, dy + 1, dx + 1].T

    w1f = np.ascontiguousarray(w1[:, 0:96].T)           # [96, 256]
    w1s = np.ascontiguousarray(w1[:, 96:168].T)         # [72, 256]
    w2k = np.concatenate([w2[:, 0:128].T, w2[:, 128:256].T], axis=1)  # [128,256]
    # head channels on-chip: partitions 0..2 = region, 32 = anom, 33 = qual
    wh = np.zeros((128, 34), np.float32)
    wh[:, 0:3] = rw.T
    wh[:, 32] = aw[0]
    wh[:, 33] = qw[0]

    # one packed bf16 weight buffer [128, WB_LEN]
    wb = np.zeros((128, WB_LEN), BF)
    wb[0:84, WB_WDX:WB_WDX + 192] = wdx2.reshape(84, 192).astype(BF)
    wb[0:96, WB_PW:WB_PW + 800] = pw.reshape(96, 800).astype(BF)
    wb[0:96, WB_W1F:WB_W1F + 256] = w1f.astype(BF)
    wb[0:72, WB_W1S:WB_W1S + 256] = w1s.astype(BF)
    wb[:, WB_W2:WB_W2 + 256] = w2k.astype(BF)
    wb[:, WB_WH:WB_WH + 34] = wh.astype(BF)
    wb[0:48, WB_ON] = np.full(48, -1.0 / 48.0, BF)

    # one packed f32 constant buffer [128, 16]
    cv = np.zeros((128, 16), np.float32)
    cv[0:96, 0] = np.concatenate([b3, b5, b7])            # fbias
    cv[0:48, 1] = thr
    cv[:, 2] = b1[0:128]
    cv[:, 3] = b1[128:256]
    cv[:, 4] = b2
    cv[0:3, 5] = rb
    cv[32, 5] = ab[0]
    cv[33, 5] = qb[0]
    cv[0:96, 6] = A_vec
    cv[0:96, 7] = Bv_vec

    shared = {'wb': wb, 'cv': cv}

    cores = []
    for i in range(8):
        b, half = i // 2, i % 2
        r0 = R * half
        xp = np.zeros((3, XP_LEN), BF)
        body = np.zeros((3, XP_ROWS + 4, WP), np.float32)
        y0, y1 = max(0, r0 - 4), min(H, r0 - 4 + XP_ROWS)
        body[:, y0 - (r0 - 4):y1 - (r0 - 4), COL0:COL0 + W] = x[b, :, y0:y1, :]
        xp[:, 8:8 + (XP_ROWS + 4) * WP] = body.reshape(3, -1).astype(BF)
        xf = np.zeros((3, 258, WP), BF)                   # zero rows 0, 257
        xf[:, 1:257, COL0:COL0 + W] = x[b].astype(BF)
        cvi = shared['cv'].copy()
        cvi[0, 8] = 1.0 if r0 > 0 else 0.0                # row masks
        cvi[0, 9] = 1.0 if r0 + R < H else 0.0
        cores.append(dict(xp=xp, xf=xf, cv=cvi, b=b, r0=r0))
    return shared, cores


# --------------------------------------------------------------------------
# device program
# --------------------------------------------------------------------------

def _build_nc():
    nc = bacc.Bacc(None, target_bir_lowering=False, debug=False)

    xp_t = nc.declare_dram_parameter("xp", [3, XP_LEN], BF16, isOutput=False)
    xf_t = nc.declare_dram_parameter("xf", [3, 258, WP], BF16, isOutput=False)
    wb_t = nc.declare_dram_parameter("wb", [128, WB_LEN], BF16, isOutput=False)
    cv_t = nc.declare_dram_parameter("cv", [128, 16], F32, isOutput=False)
    out_t = nc.declare_dram_parameter("out", [5, R, W], F32, isOutput=True)

    with tile.TileContext(nc) as tc:
        import contextlib
        with contextlib.ExitStack() as ctx:
            consts = ctx.enter_context(tc.tile_pool(name="consts", bufs=1))
            big = ctx.enter_context(tc.tile_pool(name="big", bufs=1))
            sgp = ctx.enter_context(tc.tile_pool(name="sgp", bufs=4))
            work = ctx.enter_context(tc.tile_pool(name="work", bufs=3))
            sap = ctx.enter_context(tc.tile_pool(name="sap", bufs=4))
            qpp = ctx.enter_context(tc.tile_pool(name="qpp", bufs=4))
            sob_cm = tc.tile_pool(name="sob", bufs=1)
            sob = sob_cm.__enter__()

            # ---- urgent constants first: the wdx slice of wb (phase-1
            # matmuls) and cv (phase-1 activation bias) ----
            wb_sb = consts.tile([128, WB_LEN], BF16)
            nc.sync.dma_start(out=wb_sb[:, 0:WB_PW], in_=wb_t[:, 0:WB_PW])
            cv_sb = consts.tile([128, 16], F32)
            nc.sync.dma_start(out=cv_sb, in_=cv_t[:, :])

            # ---- phase 0a: gray planes via shifted-copy DMAs ----
            g0 = sob.tile([128, 2, WP], BF16)    # gray rows r
            gu = sob.tile([128, 2, WP], BF16)    # gray rows r+1
            gd = sob.tile([128, 2, WP], BF16)    # gray rows r-1
            xsh = sob.tile([128, 3, 2, 3, WP], BF16)  # [p, shift, blk, c, w]
            for k, (sh, c) in enumerate([(s, c) for s in range(3)
                                         for c in range(3)]):
                in_ap = bass.AP(
                    tensor=xf_t[:, :, :].tensor,
                    offset=c * 258 * WP + sh * WP,
                    ap=[[WP, 128], [128 * WP, 2], [1, WP]])
                eng = (nc.sync, nc.gpsimd, nc.scalar)[k % 3]
                eng.dma_start(out=xsh[:, sh, :, c, :], in_=in_ap)
            for sh, gt in ((1, g0), (2, gu), (0, gd)):
                nc.vector.tensor_add(gt, xsh[:, sh, :, 0, :],
                                     xsh[:, sh, :, 1, :])
                nc.vector.tensor_add(gt, gt, xsh[:, sh, :, 2, :])

            # stacked-x group loader (phase 1 feed); prefetch the first two
            # groups before the bulky tail of wb lands on the sync queue
            sc_tiles = {}

            def load_sc(g):
                if g in sc_tiles:
                    return sc_tiles[g]
                gw = min(SGG, N1 - g * SGG) * CHUNK + 6
                sc = sgp.tile([84, SGG * CHUNK + 8], BF16,
                              name=f"sc{g}", tag="sc")
                for rg in range(4):
                    in_ap = bass.AP(
                        tensor=xp_t[:, :].tensor,
                        offset=8 + g * SGG * CHUNK - 3 + rg,
                        ap=[[WP, 7], [XP_LEN, 3], [1, gw]])
                    nc.sync.dma_start(
                        out=sc[21 * rg:21 * (rg + 1), 0:gw], in_=in_ap)
                sc_tiles[g] = sc
                return sc

            load_sc(0)
            load_sc(1)
            # the rest of the weight buffer (pattern/cls weights; not needed
            # until the phase-0 tail folds and phase 2)
            nc.sync.dma_start(out=wb_sb[:, WB_PW:], in_=wb_t[:, WB_PW:])
            wdx_sb = wb_sb[0:84, WB_WDX:WB_WDX + 192].rearrange(
                "p (m k) -> p m k", m=2)
            pw_sb = wb_sb[0:96, WB_PW:WB_PW + 800]
            w1f_sb = wb_sb[0:96, WB_W1F:WB_W1F + 256]
            w1s_sb = wb_sb[0:72, WB_W1S:WB_W1S + 256]
            w2_sb = wb_sb[:, WB_W2:WB_W2 + 256]
            wh_sb = wb_sb[:, WB_WH:WB_WH + 34]
            on48_sb = wb_sb[0:48, WB_ON:WB_ON + 1]
            fb_sb = cv_sb[0:96, 0:1]
            thr_sb = cv_sb[0:48, 1:2]
            b1_sb = cv_sb[:, 2:4]
            b2_sb = cv_sb[:, 4:5]
            bh_sb = cv_sb[0:34, 5:6]
            av_sb = cv_sb[0:96, 6:7]
            bv_sb = cv_sb[0:96, 7:8]
            msk_sb = cv_sb[0:1, 8:10]
            ones96 = consts.tile([1, 96], F32)
            nc.vector.memset(ones96, 1.0)
            ones128 = consts.tile([128, 1], F32)
            nc.vector.memset(ones128, 1.0)
            pwf8 = consts.tile([96, 10 * PWS], F8)     # folded pattern w (fp8)
            w1ff_sb = consts.tile([96, 256], BF16)     # folded cls1 feat weights
            m96 = consts.tile([96, 2], F32)            # row masks broadcast
            s96 = consts.tile([96, 1], F32)            # feature scale vector
            sgsc = consts.tile([1, 1], F32)            # S_g scalar

            fring = big.tile([96, RING_LEN], BF16)     # bf16 features (cls1)
            f8a = big.tile([96, F8A_LEN], F8)          # fp8 features (pattern)
            f8b = big.tile([96, F8B_LEN], F8)          # f8b[p] = f[p-1]

            with tc.tile_pool(name="ps0", bufs=1, space="PSUM") as ps0:
                # ---- phase 0b: Sobel from the gray planes (DVE only) ----
                Dt = sob.tile([128, 2, WP], BF16)
                nc.vector.tensor_sub(Dt, gu, gd)
                gy = sob.tile([128, 2, 256], BF16)
                q2 = sob.tile([128, 2, 256], BF16)
                nc.vector.tensor_add(gy, Dt[:, :, 2:258], Dt[:, :, 4:260])
                nc.vector.tensor_scalar_mul(q2, Dt[:, :, 3:259], 2.0)
                nc.vector.tensor_add(gy, gy, q2)
                cd0 = sob.tile([128, 2, 256], BF16)
                nc.vector.tensor_sub(cd0, g0[:, :, 4:260], g0[:, :, 2:258])
                gx = sob.tile([128, 2, 256], BF16)
                nc.vector.tensor_scalar_mul(gx, cd0, 2.0)
                nc.vector.tensor_sub(cd0, gu[:, :, 4:260], gu[:, :, 2:258])
                nc.vector.tensor_add(gx, gx, cd0)
                nc.vector.tensor_sub(cd0, gd[:, :, 4:260], gd[:, :, 2:258])
                nc.vector.tensor_add(gx, gx, cd0)
                nc.vector.tensor_mul(gx, gx, gx)
                nc.vector.tensor_mul(gy, gy, gy)
                nc.vector.tensor_add(gx, gx, gy)
                gsc = sob.tile([128, 2, 256], F32)
                rs = sob.tile([128, 1], F32)
                # broadcast row masks now (prologue chunk 0 reads m96)
                scr = ps0.tile([96, 2], F32, tag="scr")
                nc.tensor.matmul(scr, lhsT=ones96, rhs=msk_sb,
                                 start=True, stop=True)
                nc.vector.tensor_copy(m96, scr)

            nc.vector.memset(f8a[:, 0:G], 0.0)
            nc.vector.memset(f8b[:, 0:G + 1], 0.0)

            with tc.tile_pool(name="ppp", bufs=1, space="PSUM") as ppp, \
                 tc.tile_pool(name="ps1p", bufs=1, space="PSUM") as ps1p, \
                 tc.tile_pool(name="ps2p", bufs=1, space="PSUM") as ps2p, \
                 tc.tile_pool(name="php", bufs=2, space="PSUM") as php, \
                 tc.tile_pool(name="pfp", bufs=2, space="PSUM") as pfp:

                tail_st = {}

                def phase0_tail_a():
                    # sqrt + S_g partition-sum; scratch lives on the heads
                    # tag (no other "ph" allocations until tail_b reads it)
                    nc.scalar.activation(gsc, gx, AF.Sqrt, accum_out=rs)
                    scr2 = php.tile([96, 8], F32, name="scr2", tag="ph")
                    nc.tensor.matmul(scr2[0:1, 0:1], lhsT=rs, rhs=ones128,
                                     start=True, stop=True)
                    tail_st['scr2'] = scr2

                def phase0_tail_b():
                    scr2 = tail_st['scr2']
                    nc.vector.tensor_copy(sgsc, scr2[0:1, 0:1])
                    nc.tensor.matmul(scr2[:, 2:3], lhsT=ones96, rhs=sgsc,
                                     start=True, stop=True)
                    nc.vector.tensor_scalar(s96, scr2[:, 2:3], bv_sb, av_sb,
                                            op0=ALU.mult, op1=ALU.add)
                    nc.vector.tensor_scalar_mul(pwf8, pw_sb, s96)
                    nc.vector.tensor_scalar_mul(w1ff_sb, w1f_sb, s96)

                ph1_pend = {}

                def phase1_mm(i):
                    # phase-1 matmuls only (PE queue); the drain is issued
                    # separately so mid-chunk interleaving never inserts a
                    # Scalar op between the phase-2 drains
                    g, qq = divmod(i, SGG)
                    sc = load_sc(g)
                    pf = pfp.tile([96, CHUNK], F32, name=f"pf{i}", tag="pf")
                    for m in range(2):
                        nc.tensor.matmul(
                            pf, lhsT=wdx_sb[:, m, :],
                            rhs=sc[:, qq * CHUNK + 4 * m:
                                   qq * CHUNK + 4 * m + CHUNK],
                            start=(m == 0), stop=(m == 1))
                    ph1_pend[i] = pf

                def phase1_drain(i):
                    pf = ph1_pend.pop(i)
                    s = i * CHUNK
                    slot = (i % NB) * CHUNK
                    nc.scalar.activation(
                        out=fring[:, slot:slot + CHUNK], in_=pf,
                        func=AF.Identity, bias=fb_sb, scale=1.0)
                    if i % NB == 0 and i > 0:
                        # mirror slot-0 head at ring tail for wrap-crossing
                        # cls1 reads
                        nc.scalar.activation(
                            out=fring[:, NB * CHUNK:NB * CHUNK + WP],
                            in_=pf[:, 0:WP],
                            func=AF.Identity, bias=fb_sb, scale=1.0)
                    # fp8 shadows for the 3x3 pattern conv (casting DMAs --
                    # data moves on the DMA hw, not a compute engine).
                    # Copy chunk pairs: slots i-1, i are ring-contiguous for
                    # odd i (wrap happens at even i).
                    if i % 2 == 1:
                        s0, sl0 = s - CHUNK, slot - CHUNK
                        nc.gpsimd.dma_start(
                            out=f8a[:, G + s0:G + s0 + 2 * CHUNK],
                            in_=fring[:, sl0:sl0 + 2 * CHUNK])
                        nc.gpsimd.dma_start(
                            out=f8b[:, G + s0 + 1:G + s0 + 2 * CHUNK + 1],
                            in_=fring[:, sl0:sl0 + 2 * CHUNK])
                        # re-zero the pattern-conv pad columns of the rows
                        # touched by the pair copy above
                        row_lo = (s - CHUNK) // WP
                        row_hi = min((s + CHUNK - 1) // WP, FR - 1)
                        fva = f8a[:, G + row_lo * WP:
                                  G + (row_hi + 1) * WP].rearrange(
                            "p (r w) -> p r w", w=WP)
                        nc.gpsimd.memset(fva[:, :, 2:3], 0.0)
                        nc.gpsimd.memset(fva[:, :, 259:260], 0.0)
                        fvb = f8b[:, G + row_lo * WP + 1:
                                  G + (row_hi + 1) * WP + 1].rearrange(
                            "p (r w) -> p r w", w=WP)
                        nc.gpsimd.memset(fvb[:, :, 2:3], 0.0)
                        nc.gpsimd.memset(fvb[:, :, 259:260], 0.0)
                    if i == 1:
                        nc.vector.tensor_scalar_mul(
                            f8a[:, G:G + WP], f8a[:, G:G + WP], m96[:, 0:1])
                        nc.vector.tensor_scalar_mul(
                            f8b[:, G + 1:G + WP + 1],
                            f8b[:, G + 1:G + WP + 1], m96[:, 0:1])
                    if i == N1 - 1:
                        r129 = G + 129 * WP
                        nc.vector.tensor_scalar_mul(
                            f8a[:, r129:r129 + WP],
                            f8a[:, r129:r129 + WP], m96[:, 1:2])
                        nc.vector.tensor_scalar_mul(
                            f8b[:, r129 + 1:r129 + WP + 1],
                            f8b[:, r129 + 1:r129 + WP + 1], m96[:, 1:2])

                def phase1_chunk(i):
                    phase1_mm(i)
                    phase1_drain(i)

                def phase2_chunk(j, os_t, jl, mid=None):
                    t = WP + j * CHUNK
                    rslot = (j % NB) * CHUNK + WP
                    if j % 2 == 0:
                        # sliding fp8 tap window for the dy=+1 row (3 dx
                        # copies); dx=+-1 of dy in {-1,0} come from f8b
                        pt = qpp.tile([96, 3, PW], F8, name=f"p8{j}", tag="p8")
                        nc.gpsimd.dma_start(out=pt, in_=bass.AP(
                            tensor=f8a.tensor,
                            offset=f8a.offset + G + t + WP - 1,
                            ap=[[f8a.ap[0][0], 96], [1, 3], [1, PW]]))
                        phase2_chunk.pt = pt
                    pt = phase2_chunk.pt
                    qo = (j % 2) * CHUNK
                    pp = ppp.tile([72, CHUNK], F32, name=f"pp{j}", tag="pp")
                    pv = pwf8.rearrange("p (t m) -> p t m", t=10)
                    # 5 dual-fp8 DoubleRow passes
                    passes = (
                        (f8a, G + t - WP, WP),            # (-1,0),(0,0)
                        (f8b, G + t - WP, WP),            # (-1,-1),(0,-1)
                        (f8b, G + t - WP + 2, WP),        # (-1,1),(0,1)
                        (pt, qo, PW),                     # (1,-1),(1,0)
                        (pt, 2 * PW + qo, 16),            # (1,1), dead
                    )
                    for k, (buf, off, stride) in enumerate(passes):
                        rhs = bass.AP(
                            tensor=buf.tensor, offset=buf.offset + off,
                            ap=[[buf.ap[0][0], 96], [stride, 2], [1, CHUNK]])
                        nc.tensor.matmul(
                            pp, lhsT=pv[:, 2 * k:2 * k + 2, 0:72],
                            rhs=rhs, start=(k == 0), stop=(k == 4),
                            perf_mode=DR)
                    sa = sap.tile([72, CHUNK], BF16, name=f"sa{j}", tag="sa")
                    nc.scalar.activation(sa, pp, AF.Sigmoid)
                    an = sap.tile([48, CHUNK], BF16, name=f"an{j}", tag="an")
                    nc.vector.tensor_scalar(an, sa[0:48, :], thr_sb, 0.0,
                                            op0=ALU.subtract, op1=ALU.min)
                    ps1 = ps1p.tile([128, 2 * CHUNK], F32, name=f"ps1_{j}",
                                    tag="ps1")
                    for m in range(2):
                        nc.tensor.matmul(
                            ps1[:, m * CHUNK:(m + 1) * CHUNK],
                            lhsT=w1ff_sb[:, m * 128:(m + 1) * 128],
                            rhs=fring[:, rslot:rslot + CHUNK],
                            start=True, stop=False)
                        nc.tensor.matmul(
                            ps1[:, m * CHUNK:(m + 1) * CHUNK],
                            lhsT=w1s_sb[:, m * 128:(m + 1) * 128],
                            rhs=sa, start=False, stop=True)
                    h = work.tile([128, 2 * CHUNK], BF16, name=f"h{j}", tag="h")
                    nc.scalar.activation(h[:, 0:CHUNK], ps1[:, 0:CHUNK],
                                         AF.Relu, bias=b1_sb[:, 0:1])
                    nc.vector.tensor_scalar(h[:, CHUNK:], ps1[:, CHUNK:],
                                            b1_sb[:, 1:2], 0.0,
                                            op0=ALU.add, op1=ALU.max)
                    if mid is not None:
                        # independent phase-1 matmuls (PE-only) fill the PE
                        # while the h drains complete
                        mid()
                    # anomaly-mean next: its input (an) is ready, and with
                    # php double-buffered the tile alloc doesn't stall on the
                    # previous chunk's head drains
                    pht = php.tile([66, CHUNK], F32, name=f"ph{j}", tag="ph")
                    nc.tensor.matmul(pht[64:65, :], lhsT=on48_sb, rhs=an,
                                     start=True, stop=True)
                    ps2 = ps2p.tile([128, CHUNK], F32, name=f"ps2_{j}",
                                    tag="ps2")
                    nc.tensor.matmul(ps2, lhsT=w2_sb[:, 0:128],
                                     rhs=h[:, 0:CHUNK], start=True, stop=False)
                    nc.tensor.matmul(ps2, lhsT=w2_sb[:, 128:256],
                                     rhs=h[:, CHUNK:], start=False, stop=True)
                    p = work.tile([128, CHUNK], BF16, name=f"p{j}", tag="p")
                    nc.vector.tensor_scalar(p, ps2, b2_sb, 0.0,
                                            op0=ALU.add, op1=ALU.max)
                    nc.tensor.matmul(pht[0:34, :], lhsT=wh_sb, rhs=p,
                                     start=True, stop=True)
                    osl = os_t[:, jl * CHUNK:(jl + 1) * CHUNK]
                    nc.scalar.activation(osl[32:34, :], pht[32:34, :],
                                         AF.Sigmoid, bias=bh_sb[32:34, :])
                    nc.vector.tensor_add(osl[32:33, :], osl[32:33, :],
                                         pht[64:65, :])
                    nc.scalar.activation(osl[0:3, :], pht[0:3, :],
                                         AF.Identity, bias=bh_sb[0:3, :])

                for i in range(5):
                    phase1_chunk(i)
                phase0_tail_a()
                for i in range(5, LEAD):
                    phase1_chunk(i)
                phase0_tail_b()
                sob_cm.__exit__(None, None, None)
                osp = ctx.enter_context(tc.tile_pool(name="osp", bufs=2))
                # stream each 16-row output block as soon as its chunks
                # complete (bf16->f32 casting DMAs), so the final DMAs
                # don't all pile up after the last chunk
                blk = {8: (0, 16), 16: (16, 16), 25: (32, 16),
                       29: (48, 8), 33: (56, 8)}
                for hh in range(2):
                    os_t = osp.tile([34, OSH], BF16, name=f"os{hh}", tag="os")
                    ov = os_t.rearrange("p (r w) -> p r w", w=WP)
                    for jl in range(34):
                        j = hh * 34 + jl
                        d = j + LEAD - 1
                        if j >= 1 and LEAD <= d < N1:
                            phase1_drain(d)
                        mid = (lambda i=j + LEAD: phase1_mm(i)) \
                            if j + LEAD < N1 else None
                        phase2_chunk(j, os_t, jl, mid)
                        if jl in blk:
                            rr, nr = blk[jl]
                            nc.gpsimd.dma_start(
                                out=out_t[0:3,
                                          hh * 64 + rr:hh * 64 + rr + nr, :],
                                in_=ov[0:3, rr:rr + nr, COL0:COL0 + W])
                            nc.gpsimd.dma_start(
                                out=out_t[3:5,
                                          hh * 64 + rr:hh * 64 + rr + nr, :],
                                in_=ov[32:34, rr:rr + nr, COL0:COL0 + W])
    nc.compile()
    return nc


def _get_nc():
    if 'nc' not in _NC_CACHE:
        _NC_CACHE['nc'] = _build_nc()
    return _NC_CACHE['nc']


# --------------------------------------------------------------------------
# entry point
# --------------------------------------------------------------------------

def kernel(**inputs) -> np.ndarray:
    global LAST_RESULTS
    nc = _get_nc()
    shared, cores = _host_prep(inputs)
    in_maps = []
    for ci in cores:
        in_maps.append({'wb': shared['wb'], 'xp': ci['xp'], 'xf': ci['xf'],
                        'cv': ci['cv']})
    res = run_bass_kernel_spmd(nc, in_maps, core_ids=list(range(8)),
                               trace=bool(os.environ.get("BASS_TRACE")))
    LAST_RESULTS = res
    full = np.zeros((B, 5, H, W), np.float32)
    for i, ci in enumerate(cores):
        full[ci['b'], :, ci['r0']:ci['r0'] + R, :] = res.results[i]['out']
    return full

